# revision 7
# baseline (speedup 1.0000x reference)
"""HRR self-attention Trainium2 kernel.

Math: reference computes, per head (D=128):
    qkv = x @ W_qkv.T ; q,k,v heads
    kv  = irfft(rfft(k) * rfft(v))          # circular conv bind
    kv  = cumsum(kv, axis=seq)
    out = irfft(rfft(kv) * conj(rfft(q)))   # circular corr unbind
    y   = out @ W_o.T

The rfft/irfft along the head dim are linear maps, so they are folded into
W_qkv / W_o on the host: the device computes frequency-domain q,k,v directly
with one GEMM, does the complex bind / cumsum / unbind elementwise (the
cumsum commutes with the irfft), and applies the irfft+output projection as
a second GEMM. Zero extra FLOPs vs the plain projections.

Sharding: 8 cores = 4 batches x 2 head-groups (4 heads each). Each core
emits a partial output projection [M, S] for its batch; host sums the two
head-group partials per batch.

Frequency packing per head (D=128 -> rfft bins 0..64): lanes are packed as
64-bin halves so every elementwise op runs on full 128-partition tiles:
  re-half lane j (j=0..63):  Re X[j]
  im-half lane j:            j=0 -> Re X[64] (Nyquist), j>=1 -> Im X[j]
Generic complex multiply is wrong on lanes {0,64} of a [headA|headB] pair
chunk (DC im = Nyquist, not a true imag part); tiny fixup ops on GPSIMD
overwrite those two lanes with the correct real-only products.
"""

import numpy as np
import ml_dtypes

B, S, M, H = 4, 2048, 1024, 8
D = M // H          # 128
NB = D // 2         # 64 bins per half
SC = 512            # sequence chunk
NSC = S // SC       # 4
NMI = M // 128      # 8 contraction chunks
NCC = 12            # qkv freq channel chunks per core
NCO = 4             # U channel chunks per core

BF16 = ml_dtypes.bfloat16


# ---------------------------------------------------------------------------
# Host-side weight fusion
# ---------------------------------------------------------------------------

def _head_blocks(Wh, F):
    """Wh (D, M) spatial head weights -> (re_block, im_block) each (64, M)."""
    FW = F @ Wh  # (65, M) complex
    re = FW.real[0:NB]
    im = np.concatenate([FW.real[NB:NB + 1], FW.imag[1:NB]], axis=0)
    return re, im


def build_tables(W_qkv, W_o):
    """Per-core (WfT [1024,1536] bf16, WoG [512,1024] bf16)."""
    W_qkv = np.asarray(W_qkv, dtype=np.float64)
    W_o = np.asarray(W_o, dtype=np.float64)
    F = np.fft.rfft(np.eye(D), axis=-1).T  # (65, 128)
    Wq = W_qkv[0 * M:1 * M].reshape(H, D, M)
    Wk = W_qkv[1 * M:2 * M].reshape(H, D, M)
    Wv = W_qkv[2 * M:3 * M].reshape(H, D, M)

    # irfft basis in packed-channel order [re 0..63, nyq, im 1..63]
    n = np.arange(D)
    f = np.arange(NB)
    Gr = np.cos(2 * np.pi * np.outer(n, f) / D) / D
    Gr[:, 1:] *= 2.0
    Gnyq = np.cos(np.pi * n)[:, None] / D
    Gi = -2.0 * np.sin(2 * np.pi * np.outer(n, f) / D) / D
    G = np.concatenate([Gr, Gnyq, Gi[:, 1:]], axis=1)  # (128, 128)

    tables = []
    for core in range(8):
        g = core % 2
        heads = [4 * g + i for i in range(4)]
        chunks = []
        out_rows = []
        for pair in range(2):
            hA, hB = heads[2 * pair], heads[2 * pair + 1]
            for Wx in (Wk, Wv, Wq):
                reA, imA = _head_blocks(Wx[hA], F)
                reB, imB = _head_blocks(Wx[hB], F)
                chunks.append(np.concatenate([reA, reB], axis=0))
                chunks.append(np.concatenate([imA, imB], axis=0))
            WoGA = W_o[:, D * hA:D * (hA + 1)] @ G  # (1024, 128)
            WoGB = W_o[:, D * hB:D * (hB + 1)] @ G
            out_rows.append(np.concatenate([WoGA.T[:NB], WoGB.T[:NB]], axis=0))
            out_rows.append(np.concatenate([WoGA.T[NB:], WoGB.T[NB:]], axis=0))
        WfT = np.concatenate(chunks, axis=0).T  # (1024, 1536)
        WoG = np.concatenate(out_rows, axis=0)  # (512, 1024)
        tables.append((np.ascontiguousarray(WfT, dtype=np.float32).astype(BF16),
                       np.ascontiguousarray(WoG, dtype=np.float32).astype(BF16)))
    return tables


# ---------------------------------------------------------------------------
# Device kernel
# ---------------------------------------------------------------------------

def build_kernel(tc, xT, wf, wo, out, reps=1):
    import concourse.mybir as mybir
    from contextlib import ExitStack

    nc = tc.nc
    bf16 = mybir.dt.bfloat16
    f32 = mybir.dt.float32
    MULT = mybir.AluOpType.mult
    ADD = mybir.AluOpType.add

    with ExitStack() as ctx:
        consts = ctx.enter_context(tc.tile_pool(name="consts", bufs=1))
        xpool = ctx.enter_context(tc.tile_pool(name="xpool", bufs=1))
        wpool = ctx.enter_context(tc.tile_pool(name="wpool", bufs=1))
        qkvp = ctx.enter_context(tc.tile_pool(name="qkvp", bufs=2))
        kvp = ctx.enter_context(tc.tile_pool(name="kvp", bufs=2))
        scanp = ctx.enter_context(tc.tile_pool(name="scanp", bufs=2))
        up = ctx.enter_context(tc.tile_pool(name="up", bufs=2))
        tmpp = ctx.enter_context(tc.tile_pool(name="tmpp", bufs=3))
        outp = ctx.enter_context(tc.tile_pool(name="outp", bufs=3))
        psq = ctx.enter_context(tc.tile_pool(name="psq", bufs=3, space="PSUM"))
        psop = ctx.enter_context(tc.tile_pool(name="psop", bufs=2, space="PSUM"))

        ones = consts.tile([128, SC], bf16)
        nc.vector.memset(ones[:], 1.0)

        wf_t = []
        for mi in range(NMI):
            t = wpool.tile([128, 1536], bf16, tag=f"wf{mi}", name=f"wf{mi}")
            nc.sync.dma_start(out=t[:], in_=wf[mi * 128:(mi + 1) * 128, :])
            wf_t.append(t)
        wo_t = []
        for ci in range(NCO):
            t = wpool.tile([128, 1024], bf16, tag=f"wo{ci}", name=f"wo{ci}")
            nc.sync.dma_start(out=t[:], in_=wo[ci * 128:(ci + 1) * 128, :])
            wo_t.append(t)
        x_t = [[None] * NSC for _ in range(NMI)]
        for sc in range(NSC):
            for mi in range(NMI):
                t = xpool.tile([128, SC], bf16, tag=f"x{mi}_{sc}", name=f"x{mi}_{sc}")
                nc.sync.dma_start(
                    out=t[:], in_=xT[mi * 128:(mi + 1) * 128, sc * SC:(sc + 1) * SC])
                x_t[mi][sc] = t

        for rep in range(reps):
          prev_scan = {}
          for sc in range(NSC):
            chunks = []
            for cc in range(NCC):
                ps = psq.tile([128, SC], f32, tag="psq", name=f"psq{rep}_{sc}_{cc}")
                for mi in range(NMI):
                    nc.tensor.matmul(
                        ps[:], wf_t[mi][:, cc * 128:(cc + 1) * 128],
                        x_t[mi][sc][:], start=(mi == 0), stop=(mi == NMI - 1))
                sb = qkvp.tile([128, SC], bf16, tag=f"qkv{cc}", name=f"qkv{sc}_{cc}")
                nc.any.tensor_copy(sb[:], ps[:])
                chunks.append(sb)

            U = []
            for pair in range(2):
                Kre, Kim, Vre, Vim, Qre, Qim = chunks[6 * pair:6 * pair + 6]
                # lanes {0, 64}: DC / Nyquist real-only fixups (strided
                # partition APs are rejected by the verifier -> one op each)
                fixes = [(slice(0, 1), slice(None)), (slice(64, 65), slice(None))]

                t1 = tmpp.tile([128, SC], bf16, tag="t1", name=f"t1_{sc}_{pair}")
                t2 = tmpp.tile([128, SC], bf16, tag="t2", name=f"t2_{sc}_{pair}")
                KVre = kvp.tile([128, SC], bf16, tag=f"kvre{pair}", name=f"kvre{sc}_{pair}")
                KVim = kvp.tile([128, SC], bf16, tag=f"kvim{pair}", name=f"kvim{sc}_{pair}")
                nc.vector.tensor_mul(t1[:], Kre[:], Vre[:])
                nc.vector.tensor_mul(t2[:], Kim[:], Vim[:])
                nc.vector.tensor_sub(KVre[:], t1[:], t2[:])
                t3 = tmpp.tile([128, SC], bf16, tag="t3", name=f"t3_{sc}_{pair}")
                t4 = tmpp.tile([128, SC], bf16, tag="t4", name=f"t4_{sc}_{pair}")
                nc.vector.tensor_mul(t3[:], Kre[:], Vim[:])
                nc.vector.tensor_mul(t4[:], Kim[:], Vre[:])
                nc.vector.tensor_add(KVim[:], t3[:], t4[:])
                for fix in fixes:
                    nc.gpsimd.tensor_mul(KVre[fix], Kre[fix], Vre[fix])
                    nc.gpsimd.tensor_mul(KVim[fix], Kim[fix], Vim[fix])

                KVre_c = scanp.tile([128, SC], f32, tag=f"scre{pair}", name=f"scre{sc}_{pair}")
                KVim_c = scanp.tile([128, SC], f32, tag=f"scim{pair}", name=f"scim{sc}_{pair}")
                init_re = 0.0 if sc == 0 else prev_scan[(pair, 0)][:, SC - 1:SC]
                init_im = 0.0 if sc == 0 else prev_scan[(pair, 1)][:, SC - 1:SC]
                nc.vector.tensor_tensor_scan(
                    KVre_c[:], ones[:], KVre[:], init_re, MULT, ADD)
                nc.vector.tensor_tensor_scan(
                    KVim_c[:], ones[:], KVim[:], init_im, MULT, ADD)
                prev_scan[(pair, 0)] = KVre_c
                prev_scan[(pair, 1)] = KVim_c

                u1 = tmpp.tile([128, SC], f32, tag="u1", name=f"u1_{sc}_{pair}")
                u2 = tmpp.tile([128, SC], f32, tag="u2", name=f"u2_{sc}_{pair}")
                Ure = up.tile([128, SC], bf16, tag=f"ure{pair}", name=f"ure{sc}_{pair}")
                Uim = up.tile([128, SC], bf16, tag=f"uim{pair}", name=f"uim{sc}_{pair}")
                nc.vector.tensor_mul(u1[:], KVre_c[:], Qre[:])
                nc.vector.tensor_mul(u2[:], KVim_c[:], Qim[:])
                nc.vector.tensor_add(Ure[:], u1[:], u2[:])
                u3 = tmpp.tile([128, SC], f32, tag="u3", name=f"u3_{sc}_{pair}")
                u4 = tmpp.tile([128, SC], f32, tag="u4", name=f"u4_{sc}_{pair}")
                nc.vector.tensor_mul(u3[:], KVim_c[:], Qre[:])
                nc.vector.tensor_mul(u4[:], KVre_c[:], Qim[:])
                nc.vector.tensor_sub(Uim[:], u3[:], u4[:])
                for fix in fixes:
                    nc.gpsimd.tensor_mul(Ure[fix], KVre_c[fix], Qre[fix])
                    nc.gpsimd.tensor_mul(Uim[fix], KVim_c[fix], Qim[fix])
                U += [Ure, Uim]

            for mo in range(8):
                po = psop.tile([128, SC], f32, tag="pso", name=f"pso{sc}_{mo}")
                for ci in range(NCO):
                    nc.tensor.matmul(
                        po[:], wo_t[ci][:, mo * 128:(mo + 1) * 128], U[ci][:],
                        start=(ci == 0), stop=(ci == NCO - 1))
                so = outp.tile([128, SC], f32, tag="so", name=f"so{sc}_{mo}")
                nc.any.tensor_copy(so[:], po[:])
                nc.sync.dma_start(
                    out=out[mo * 128:(mo + 1) * 128, sc * SC:(sc + 1) * SC],
                    in_=so[:])


def build_bass(reps=1):
    import concourse.bacc as bacc
    import concourse.tile as tile
    import concourse.mybir as mybir

    nc = bacc.Bacc("TRN2", target_bir_lowering=False, debug=False, num_devices=8)
    xT = nc.dram_tensor("xT", [M, S], mybir.dt.bfloat16, kind="ExternalInput")
    wf = nc.dram_tensor("wf", [M, 1536], mybir.dt.bfloat16, kind="ExternalInput")
    wo = nc.dram_tensor("wo", [512, M], mybir.dt.bfloat16, kind="ExternalInput")
    out = nc.dram_tensor("out", [M, S], mybir.dt.float32, kind="ExternalOutput")
    with tile.TileContext(nc) as tc:
        build_kernel(tc, xT[:], wf[:], wo[:], out[:], reps=reps)
    nc.compile()
    return nc


_NC_CACHE = {}


def _get_nc(reps=1):
    if reps not in _NC_CACHE:
        _NC_CACHE[reps] = build_bass(reps)
    return _NC_CACHE[reps]


def make_in_maps(x, W_qkv, W_o):
    tables = build_tables(W_qkv, W_o)
    x = np.asarray(x, dtype=np.float32)
    in_maps = []
    for core in range(8):
        b = core // 2
        xT_c = np.ascontiguousarray(x[b].T).astype(BF16)
        WfT, WoG = tables[core]
        in_maps.append({"xT": xT_c, "wf": WfT, "wo": WoG})
    return in_maps


def combine_outputs(results):
    out = np.empty((B, S, M), dtype=np.float32)
    for b in range(B):
        acc = results[2 * b]["out"].astype(np.float32) + \
            results[2 * b + 1]["out"].astype(np.float32)
        out[b] = acc.T
    return out


def kernel(x, W_qkv, W_o):
    from concourse.bass_utils import run_bass_kernel_spmd
    nc = _get_nc()
    in_maps = make_in_maps(x, W_qkv, W_o)
    res = run_bass_kernel_spmd(nc, in_maps, core_ids=list(range(8)))
    return combine_outputs(res.results)


# revision 11
# speedup vs baseline: 974.1287x; 974.1287x over previous
"""HRR self-attention Trainium2 kernel.

Math: reference computes, per head (D=128):
    qkv = x @ W_qkv.T ; q,k,v heads
    kv  = irfft(rfft(k) * rfft(v))          # circular conv bind
    kv  = cumsum(kv, axis=seq)
    out = irfft(rfft(kv) * conj(rfft(q)))   # circular corr unbind
    y   = out @ W_o.T

The rfft/irfft along the head dim are linear maps, so they are folded into
W_qkv / W_o on the host: the device computes frequency-domain q,k,v directly
with one GEMM, does the complex bind / cumsum / unbind elementwise (the
cumsum commutes with the irfft), and applies the irfft+output projection as
a second GEMM. Zero extra FLOPs vs the plain projections.

Sharding: 8 cores = 4 batches x 2 head-groups (4 heads each). Each core
emits a partial output projection [M, S] for its batch; host sums the two
head-group partials per batch.

Frequency packing per head (D=128 -> rfft bins 0..64): lanes are packed as
64-bin halves so every elementwise op runs on full 128-partition tiles:
  re-half lane j (j=0..63):  Re X[j]
  im-half lane j:            j=0 -> Re X[64] (Nyquist), j>=1 -> Im X[j]
Generic complex multiply is wrong on lanes {0,64} of a [headA|headB] pair
chunk (DC im = Nyquist, not a true imag part); tiny fixup ops on GPSIMD
overwrite those two lanes with the correct real-only products.
"""

import numpy as np
import ml_dtypes

B, S, M, H = 4, 2048, 1024, 8
D = M // H          # 128
NB = D // 2         # 64 bins per half
SC = 512            # sequence chunk
NSC = S // SC       # 4
NMI = M // 128      # 8 contraction chunks
NCC = 12            # qkv freq channel chunks per core
NCO = 4             # U channel chunks per core

BF16 = ml_dtypes.bfloat16


# ---------------------------------------------------------------------------
# Host-side weight fusion
# ---------------------------------------------------------------------------

def _head_blocks(Wh, F):
    """Wh (D, M) spatial head weights -> (re_block, im_block) each (64, M)."""
    FW = F @ Wh  # (65, M) complex
    re = FW.real[0:NB]
    im = np.concatenate([FW.real[NB:NB + 1], FW.imag[1:NB]], axis=0)
    return re, im


def build_tables(W_qkv, W_o):
    """Per-core (WfT [1024,1536] bf16, WoG [512,1024] bf16)."""
    W_qkv = np.asarray(W_qkv, dtype=np.float64)
    W_o = np.asarray(W_o, dtype=np.float64)
    F = np.fft.rfft(np.eye(D), axis=-1).T  # (65, 128)
    Wq = W_qkv[0 * M:1 * M].reshape(H, D, M)
    Wk = W_qkv[1 * M:2 * M].reshape(H, D, M)
    Wv = W_qkv[2 * M:3 * M].reshape(H, D, M)

    # irfft basis in packed-channel order [re 0..63, nyq, im 1..63]
    n = np.arange(D)
    f = np.arange(NB)
    Gr = np.cos(2 * np.pi * np.outer(n, f) / D) / D
    Gr[:, 1:] *= 2.0
    Gnyq = np.cos(np.pi * n)[:, None] / D
    Gi = -2.0 * np.sin(2 * np.pi * np.outer(n, f) / D) / D
    G = np.concatenate([Gr, Gnyq, Gi[:, 1:]], axis=1)  # (128, 128)

    tables = []
    for core in range(8):
        g = core % 2
        heads = [4 * g + i for i in range(4)]
        chunks = []
        out_rows = []
        for pair in range(2):
            hA, hB = heads[2 * pair], heads[2 * pair + 1]
            for Wx in (Wk, Wv, Wq):
                reA, imA = _head_blocks(Wx[hA], F)
                reB, imB = _head_blocks(Wx[hB], F)
                chunks.append(np.concatenate([reA, reB], axis=0))
                chunks.append(np.concatenate([imA, imB], axis=0))
            WoGA = W_o[:, D * hA:D * (hA + 1)] @ G  # (1024, 128)
            WoGB = W_o[:, D * hB:D * (hB + 1)] @ G
            out_rows.append(np.concatenate([WoGA.T[:NB], WoGB.T[:NB]], axis=0))
            out_rows.append(np.concatenate([WoGA.T[NB:], WoGB.T[NB:]], axis=0))
        WfT = np.concatenate(chunks, axis=0).T  # (1024, 1536)
        WoG = np.concatenate(out_rows, axis=0)  # (512, 1024)
        tables.append((np.ascontiguousarray(WfT, dtype=np.float32).astype(BF16),
                       np.ascontiguousarray(WoG, dtype=np.float32).astype(BF16)))
    return tables


# ---------------------------------------------------------------------------
# Device kernel
# ---------------------------------------------------------------------------

def build_kernel(tc, xT, wf, wo, out, reps=1, loop_iters=None):
    import concourse.mybir as mybir
    from contextlib import ExitStack

    nc = tc.nc
    bf16 = mybir.dt.bfloat16
    f32 = mybir.dt.float32
    MULT = mybir.AluOpType.mult
    ADD = mybir.AluOpType.add

    with ExitStack() as ctx:
        consts = ctx.enter_context(tc.tile_pool(name="consts", bufs=1))
        xpool = ctx.enter_context(tc.tile_pool(name="xpool", bufs=1))
        wpool = ctx.enter_context(tc.tile_pool(name="wpool", bufs=1))
        qkvp = ctx.enter_context(tc.tile_pool(name="qkvp", bufs=2))
        kvp = ctx.enter_context(tc.tile_pool(name="kvp", bufs=2))
        scanp = ctx.enter_context(tc.tile_pool(name="scanp", bufs=2))
        up = ctx.enter_context(tc.tile_pool(name="up", bufs=2))
        tmpp = ctx.enter_context(tc.tile_pool(name="tmpp", bufs=3))
        outp = ctx.enter_context(tc.tile_pool(name="outp", bufs=3))
        psq = ctx.enter_context(tc.tile_pool(name="psq", bufs=3, space="PSUM"))
        psop = ctx.enter_context(tc.tile_pool(name="psop", bufs=2, space="PSUM"))

        ones = consts.tile([128, SC], bf16)
        nc.vector.memset(ones[:], 1.0)

        wf_t = []
        for mi in range(NMI):
            t = wpool.tile([128, 1536], bf16, tag=f"wf{mi}", name=f"wf{mi}")
            nc.sync.dma_start(out=t[:], in_=wf[mi * 128:(mi + 1) * 128, :])
            wf_t.append(t)
        wo_t = []
        for ci in range(NCO):
            t = wpool.tile([128, 1024], bf16, tag=f"wo{ci}", name=f"wo{ci}")
            nc.sync.dma_start(out=t[:], in_=wo[ci * 128:(ci + 1) * 128, :])
            wo_t.append(t)
        x_t = [[None] * NSC for _ in range(NMI)]
        for sc in range(NSC):
            for mi in range(NMI):
                t = xpool.tile([128, SC], bf16, tag=f"x{mi}_{sc}", name=f"x{mi}_{sc}")
                nc.sync.dma_start(
                    out=t[:], in_=xT[mi * 128:(mi + 1) * 128, sc * SC:(sc + 1) * SC])
                x_t[mi][sc] = t

        if loop_iters is not None:
            loop_cm = tc.For_i(
                0, loop_iters, 1,
                hint_engines=(mybir.EngineType.PE, mybir.EngineType.DVE,
                              mybir.EngineType.Activation, mybir.EngineType.Pool,
                              mybir.EngineType.SP))
            loop_cm.__enter__()
        for rep in range(reps):
          prev_scan = {}
          for sc in range(NSC):
            chunks = []
            for cc in range(NCC):
                ps = psq.tile([128, SC], f32, tag="psq", name=f"psq{rep}_{sc}_{cc}")
                for mi in range(NMI):
                    nc.tensor.matmul(
                        ps[:], wf_t[mi][:, cc * 128:(cc + 1) * 128],
                        x_t[mi][sc][:], start=(mi == 0), stop=(mi == NMI - 1))
                sb = qkvp.tile([128, SC], bf16, tag=f"qkv{cc}", name=f"qkv{sc}_{cc}")
                nc.any.tensor_copy(sb[:], ps[:])
                chunks.append(sb)

            U = []
            for pair in range(2):
                Kre, Kim, Vre, Vim, Qre, Qim = chunks[6 * pair:6 * pair + 6]
                # lanes {0, 64}: DC / Nyquist real-only fixups (strided
                # partition APs are rejected by the verifier -> one op each)
                fixes = [(slice(0, 1), slice(None)), (slice(64, 65), slice(None))]

                t1 = tmpp.tile([128, SC], bf16, tag="t1", name=f"t1_{sc}_{pair}")
                t2 = tmpp.tile([128, SC], bf16, tag="t2", name=f"t2_{sc}_{pair}")
                KVre = kvp.tile([128, SC], bf16, tag=f"kvre{pair}", name=f"kvre{sc}_{pair}")
                KVim = kvp.tile([128, SC], bf16, tag=f"kvim{pair}", name=f"kvim{sc}_{pair}")
                nc.vector.tensor_mul(t1[:], Kre[:], Vre[:])
                nc.vector.tensor_mul(t2[:], Kim[:], Vim[:])
                nc.vector.tensor_sub(KVre[:], t1[:], t2[:])
                t3 = tmpp.tile([128, SC], bf16, tag="t3", name=f"t3_{sc}_{pair}")
                t4 = tmpp.tile([128, SC], bf16, tag="t4", name=f"t4_{sc}_{pair}")
                nc.vector.tensor_mul(t3[:], Kre[:], Vim[:])
                nc.vector.tensor_mul(t4[:], Kim[:], Vre[:])
                nc.vector.tensor_add(KVim[:], t3[:], t4[:])
                for fix in fixes:
                    nc.gpsimd.tensor_mul(KVre[fix], Kre[fix], Vre[fix])
                    nc.gpsimd.tensor_mul(KVim[fix], Kim[fix], Vim[fix])

                KVre_c = scanp.tile([128, SC], f32, tag=f"scre{pair}", name=f"scre{sc}_{pair}")
                KVim_c = scanp.tile([128, SC], f32, tag=f"scim{pair}", name=f"scim{sc}_{pair}")
                init_re = 0.0 if sc == 0 else prev_scan[(pair, 0)][:, SC - 1:SC]
                init_im = 0.0 if sc == 0 else prev_scan[(pair, 1)][:, SC - 1:SC]
                nc.vector.tensor_tensor_scan(
                    KVre_c[:], ones[:], KVre[:], init_re, MULT, ADD)
                nc.vector.tensor_tensor_scan(
                    KVim_c[:], ones[:], KVim[:], init_im, MULT, ADD)
                prev_scan[(pair, 0)] = KVre_c
                prev_scan[(pair, 1)] = KVim_c

                u1 = tmpp.tile([128, SC], f32, tag="u1", name=f"u1_{sc}_{pair}")
                u2 = tmpp.tile([128, SC], f32, tag="u2", name=f"u2_{sc}_{pair}")
                Ure = up.tile([128, SC], bf16, tag=f"ure{pair}", name=f"ure{sc}_{pair}")
                Uim = up.tile([128, SC], bf16, tag=f"uim{pair}", name=f"uim{sc}_{pair}")
                nc.vector.tensor_mul(u1[:], KVre_c[:], Qre[:])
                nc.vector.tensor_mul(u2[:], KVim_c[:], Qim[:])
                nc.vector.tensor_add(Ure[:], u1[:], u2[:])
                u3 = tmpp.tile([128, SC], f32, tag="u3", name=f"u3_{sc}_{pair}")
                u4 = tmpp.tile([128, SC], f32, tag="u4", name=f"u4_{sc}_{pair}")
                nc.vector.tensor_mul(u3[:], KVim_c[:], Qre[:])
                nc.vector.tensor_mul(u4[:], KVre_c[:], Qim[:])
                nc.vector.tensor_sub(Uim[:], u3[:], u4[:])
                for fix in fixes:
                    nc.gpsimd.tensor_mul(Ure[fix], KVre_c[fix], Qre[fix])
                    nc.gpsimd.tensor_mul(Uim[fix], KVim_c[fix], Qim[fix])
                U += [Ure, Uim]

            for mo in range(8):
                po = psop.tile([128, SC], f32, tag="pso", name=f"pso{sc}_{mo}")
                for ci in range(NCO):
                    nc.tensor.matmul(
                        po[:], wo_t[ci][:, mo * 128:(mo + 1) * 128], U[ci][:],
                        start=(ci == 0), stop=(ci == NCO - 1))
                so = outp.tile([128, SC], f32, tag="so", name=f"so{sc}_{mo}")
                nc.any.tensor_copy(so[:], po[:])
                nc.sync.dma_start(
                    out=out[mo * 128:(mo + 1) * 128, sc * SC:(sc + 1) * SC],
                    in_=so[:])
        if loop_iters is not None:
            loop_cm.__exit__(None, None, None)


def build_bass(reps=1, loop_iters=None):
    import concourse.bacc as bacc
    import concourse.tile as tile
    import concourse.mybir as mybir

    nc = bacc.Bacc("TRN2", target_bir_lowering=False, debug=False, num_devices=8)
    xT = nc.dram_tensor("xT", [M, S], mybir.dt.bfloat16, kind="ExternalInput")
    wf = nc.dram_tensor("wf", [M, 1536], mybir.dt.bfloat16, kind="ExternalInput")
    wo = nc.dram_tensor("wo", [512, M], mybir.dt.bfloat16, kind="ExternalInput")
    out = nc.dram_tensor("out", [M, S], mybir.dt.float32, kind="ExternalOutput")
    with tile.TileContext(nc) as tc:
        build_kernel(tc, xT[:], wf[:], wo[:], out[:], reps=reps,
                     loop_iters=loop_iters)
    nc.compile()
    return nc


_NC_CACHE = {}


def _get_nc(reps=1, loop_iters=None):
    key = (reps, loop_iters)
    if key not in _NC_CACHE:
        _NC_CACHE[key] = build_bass(reps, loop_iters)
    return _NC_CACHE[key]


def make_in_maps(x, W_qkv, W_o):
    tables = build_tables(W_qkv, W_o)
    x = np.asarray(x, dtype=np.float32)
    in_maps = []
    for core in range(8):
        b = core // 2
        xT_c = np.ascontiguousarray(x[b].T).astype(BF16)
        WfT, WoG = tables[core]
        in_maps.append({"xT": xT_c, "wf": WfT, "wo": WoG})
    return in_maps


def combine_outputs(results):
    out = np.empty((B, S, M), dtype=np.float32)
    for b in range(B):
        acc = results[2 * b]["out"].astype(np.float32) + \
            results[2 * b + 1]["out"].astype(np.float32)
        out[b] = acc.T
    return out


def kernel(x, W_qkv, W_o):
    from concourse.bass_utils import run_bass_kernel_spmd
    nc = _get_nc()
    in_maps = make_in_maps(x, W_qkv, W_o)
    res = run_bass_kernel_spmd(nc, in_maps, core_ids=list(range(8)))
    return combine_outputs(res.results)
